# revision 6
# baseline (speedup 1.0000x reference)
"""Trainium2 Bass kernel for the GNN message-passing model (nn_NCB_76965813944530).

Math: the reference's N x N "momentum attention matrix" is rank-1
(mam = s s^T, s = sigmoid(aw)), and gcn_norm replaces its diagonal with 1.
So   A = s s^T + diag(1 - s^2),
     colsum_j = s_j (S - s_j) + 1           with S = sum(s)
     dinv = rsqrt(colsum), An = diag(dinv) A diag(dinv)
     An @ M = g (g^T M) + diag(e) M          g = s*dinv, e = dinv^2 (1-s^2)
and  g^T (z @ W) = (g^T z) @ W, so each GCN layer needs only a [1,H]
cross-core AllReduce of t = g^T z instead of an N x N matmul.

Sharding: rows (nodes) split 1024 per core across 8 cores. Weights
replicated. One AllGather of s (for S, and for the mam = s s^T output
rows), then one tiny AllReduce per GCN layer.

The kernel is self-contained: full inputs in, full outputs out.
"""

import os
import sys

for _p in ("/opt/trn_rl_repo", "/root/.axon_site/_ro/trn_rl_repo"):
    if os.path.isdir(_p) and _p not in sys.path:
        sys.path.append(_p)

import numpy as np

import concourse.bass as bass
import concourse.mybir as mybir
import concourse.tile as tile
from concourse import bacc
from concourse.bass_utils import run_bass_kernel_spmd
from concourse.masks import make_identity

N, IN, H, OUT = 8192, 2048, 512, 128
NCORES = 8
ROWS = N // NCORES      # 1024 rows per core
MT = ROWS // 128        # 8 row tiles per core
KIN = IN // 128         # 16 contraction chunks for x @ p1_w
KH = H // 128           # 4 contraction chunks for H-dim matmuls
MAMC = 4                # mam written in 4 column chunks of 2048
MCW = N // MAMC         # 2048
EPS = 1e-5

F32 = mybir.dt.float32
MMDT = mybir.dt.float32  # dtype for DMA-fed matmul operands (float32r = fast)
AF = mybir.ActivationFunctionType
ALU = mybir.AluOpType
RG = [list(range(NCORES))]

_CACHE = {}


def _ln_inplace(nc, stat, t, g_bc, b_bc, eps_col):
    """LayerNorm over the free dim of t ([128, F]), in place, then *g + b."""
    f = t.shape[-1]
    stats = stat.tile([128, 6], F32, tag="st6")
    nc.vector.bn_stats(out=stats, in_=t)
    mv = stat.tile([128, 2], F32, tag="st2")
    nc.vector.bn_aggr(out=mv, in_=stats)
    sd = stat.tile([128, 1], F32, tag="st1")
    nc.scalar.activation(out=sd, in_=mv[:, 1:2], func=AF.Sqrt,
                         bias=eps_col, scale=1.0)
    rs = stat.tile([128, 1], F32, tag="st1b")
    nc.vector.reciprocal(out=rs, in_=sd)
    nc.vector.tensor_scalar(out=t, in0=t, scalar1=mv[:, 0:1], scalar2=rs,
                            op0=ALU.subtract, op1=ALU.mult)
    nc.vector.tensor_mul(out=t, in0=t, in1=g_bc)
    nc.vector.tensor_add(out=t, in0=t, in1=b_bc)


def _build():
    nc = bacc.Bacc("TRN2", num_devices=NCORES, debug=False)

    d_in = {}

    def din(name, shape, dt=MMDT):
        d_in[name] = nc.dram_tensor(name, shape, dt, kind="ExternalInput")
        return d_in[name]

    xT = din("xT", [IN, ROWS])
    p1w = din("p1_w", [IN, H])
    p2w = din("p2_w", [H, H])
    a1w = din("a1_w", [H, H])
    a2w = din("a2_w", [H, H])
    c1w = din("c1_w", [H, H])
    c2w = din("c2_w", [H, H])
    c3w = din("c3_w", [H, OUT])
    resw = din("res_w", [H, OUT])
    a3w = din("a3_w", [1, H], F32)
    p1b = din("p1_b", [1, H])
    p2b = din("p2_b", [1, H])
    a1b = din("a1_b", [1, H])
    a2b = din("a2_b", [1, H])
    c1b = din("c1_b", [1, H])
    c2b = din("c2_b", [1, H])
    c3b = din("c3_b", [1, OUT])
    resb = din("res_b", [1, OUT])
    a3b = din("a3_b", [1, 1], F32)
    ln0g = din("ln0_g", [1, H], F32)
    ln0b = din("ln0_b", [1, H], F32)
    ln1g = din("ln1_g", [1, H], F32)
    ln1b = din("ln1_b", [1, H], F32)
    ln2g = din("ln2_g", [1, H], F32)
    ln2b = din("ln2_b", [1, H], F32)
    ln3g = din("ln3_g", [1, OUT], F32)
    ln3b = din("ln3_b", [1, OUT], F32)

    aw_o = nc.dram_tensor("aw_o", [ROWS, 1], F32, kind="ExternalOutput")
    mam_o = nc.dram_tensor("mam_o", [ROWS, N], F32, kind="ExternalOutput")
    x3_o = nc.dram_tensor("x3_o", [ROWS, OUT], F32, kind="ExternalOutput")

    with tile.TileContext(nc) as tc:
        with (
            tc.tile_pool(name="consts", bufs=1) as consts,
            tc.tile_pool(name="wpool", bufs=1) as wp,
            tc.tile_pool(name="zgen", bufs=2) as zg,
            tc.tile_pool(name="zT", bufs=2) as ztp,
            tc.tile_pool(name="awork", bufs=4) as awork,
            tc.tile_pool(name="stat", bufs=4) as stat,
            tc.tile_pool(name="cols", bufs=1) as cols,
            tc.tile_pool(name="rows", bufs=1) as rows,
            tc.tile_pool(name="ps", bufs=8, space="PSUM") as ps,
            tc.tile_pool(name="dram", bufs=1, space="DRAM") as dram,
        ):
            # ---------------- constants / weights ----------------
            ident = consts.tile([128, 128], F32)
            make_identity(nc, ident)
            ones_row = consts.tile([1, 128], MMDT)
            nc.vector.memset(ones_row, 1.0)
            ones_col = consts.tile([128, 1], F32)
            nc.vector.memset(ones_col, 1.0)
            eps_col = consts.tile([128, 1], F32)
            nc.vector.memset(eps_col, EPS)

            def brow(dt_tensor, f, dt=MMDT):
                t = consts.tile([1, f], dt, tag=f"r_{dt_tensor.name}")
                nc.sync.dma_start(out=t, in_=dt_tensor.ap())
                return t

            def bbc(dt_tensor, f):
                t = consts.tile([128, f], F32, tag=f"b_{dt_tensor.name}")
                nc.sync.dma_start(out=t, in_=dt_tensor.ap().broadcast_to([128, f]))
                return t

            p1b_r = brow(p1b, H)
            p2b_r = brow(p2b, H)
            a1b_r = brow(a1b, H)
            a2b_r = brow(a2b, H)
            c1b_r = brow(c1b, H)
            c2b_r = brow(c2b, H)
            c3b_r = brow(c3b, OUT)
            resb_r = brow(resb, OUT)
            a3w_bc = bbc(a3w, H)
            a3b_c = bbc(a3b, 1)
            ln0g_bc = bbc(ln0g, H)
            ln0b_bc = bbc(ln0b, H)
            ln1g_bc = bbc(ln1g, H)
            ln1b_bc = bbc(ln1b, H)
            ln2g_bc = bbc(ln2g, H)
            ln2b_bc = bbc(ln2b, H)
            ln3g_bc = bbc(ln3g, OUT)
            ln3b_bc = bbc(ln3b, OUT)

            def wload(dt_tensor, fout):
                t = wp.tile([128, KH, fout], MMDT, tag=f"w_{dt_tensor.name}")
                nc.sync.dma_start(
                    out=t, in_=dt_tensor.ap().rearrange("(j p) n -> p j n", p=128))
                return t

            p2w_sb = wload(p2w, H)
            a1w_sb = wload(a1w, H)
            a2w_sb = wload(a2w, H)
            c1w_sb = wload(c1w, H)
            c2w_sb = wload(c2w, H)
            c3w_sb = wload(c3w, OUT)
            resw_sb = wload(resw, OUT)

            # dram bounce buffers for collectives
            s_bounce = dram.tile([ROWS, 1], F32)
            s_full = dram.tile([N, 1], F32)
            g_bounce = dram.tile([ROWS, 1], F32)
            t_bnc = [dram.tile([1, H], F32, tag=f"tb{i}", name=f"t_bnc{i}")
                     for i in range(3)]
            t_red = [dram.tile([1, H], F32, tag=f"tr{i}", name=f"t_red{i}")
                     for i in range(3)]

            # per-row column vectors, one column per m-tile
            s_cols = cols.tile([128, MT], F32)
            g_cols = cols.tile([128, MT], F32)
            e_cols = cols.tile([128, MT], F32)

            h_all = zg.tile([128, MT, H], F32, tag="zgen")

            # ---------------- stage A: x @ p1_w (+b, relu, LN0) ----------------
            psA = [ps.tile([128, 512], F32, tag="ps", name=f"psA{m}")
                   for m in range(MT)]
            with (
                tc.tile_pool(name="xa", bufs=4) as xa,
                tc.tile_pool(name="p1p", bufs=4) as p1p,
            ):
                for k in range(KIN):
                    xt = xa.tile([128, ROWS], MMDT, tag="xt")
                    nc.sync.dma_start(out=xt, in_=xT.ap()[k * 128:(k + 1) * 128, :])
                    wt = p1p.tile([128, H], MMDT, tag="p1w")
                    nc.sync.dma_start(out=wt, in_=p1w.ap()[k * 128:(k + 1) * 128, :])
                    for m in range(MT):
                        nc.tensor.matmul(psA[m], lhsT=xt[:, m * 128:(m + 1) * 128],
                                         rhs=wt, start=(k == 0), stop=False)
                for m in range(MT):
                    nc.tensor.matmul(psA[m], lhsT=ones_row, rhs=p1b_r,
                                     start=False, stop=True)

                lnh0T = ztp.tile([128, KH, ROWS], F32, tag="zT")
                hT = ztp.tile([128, KH, ROWS], F32, tag="zT")

                for m in range(MT):
                    ms = slice(m * 128, (m + 1) * 128)
                    # bias already in psum; relu copies psum -> sbuf
                    lnh0 = awork.tile([128, H], F32, tag="aw")
                    nc.scalar.activation(out=lnh0, in_=psA[m], func=AF.Relu)
                    _ln_inplace(nc, stat, lnh0, ln0g_bc, ln0b_bc, eps_col)
                    for j in range(KH):
                        pst = ps.tile([128, 512], F32, tag="ps")
                        nc.tensor.transpose(pst[:, 0:128],
                                            lnh0[:, j * 128:(j + 1) * 128], ident)
                        nc.scalar.copy(out=lnh0T[:, j, ms], in_=pst[:, 0:128])

                    # ---------------- stage B: h = LN0 @ p2_w + b ----------------
                    psB = ps.tile([128, 512], F32, tag="ps")
                    for j in range(KH):
                        nc.tensor.matmul(psB, lhsT=lnh0T[:, j, ms],
                                         rhs=p2w_sb[:, j, :],
                                         start=(j == 0), stop=False)
                    nc.tensor.matmul(psB, lhsT=ones_row, rhs=p2b_r,
                                     start=False, stop=True)
                    hcol = h_all[:, m, :]
                    nc.scalar.copy(out=hcol, in_=psB)
                    for j in range(KH):
                        pst = ps.tile([128, 512], F32, tag="ps")
                        nc.tensor.transpose(pst[:, 0:128],
                                            hcol[:, j * 128:(j + 1) * 128], ident)
                        nc.scalar.copy(out=hT[:, j, ms], in_=pst[:, 0:128])

                    # ---------------- attention gate -> aw, s ----------------
                    ps1 = ps.tile([128, 512], F32, tag="ps")
                    for j in range(KH):
                        nc.tensor.matmul(ps1, lhsT=hT[:, j, ms],
                                         rhs=a1w_sb[:, j, :],
                                         start=(j == 0), stop=False)
                    nc.tensor.matmul(ps1, lhsT=ones_row, rhs=a1b_r,
                                     start=False, stop=True)
                    sig = awork.tile([128, H], F32, tag="aw")
                    nc.scalar.activation(out=sig, in_=ps1, func=AF.Sigmoid)
                    ps2 = ps.tile([128, 512], F32, tag="ps")
                    for j in range(KH):
                        nc.tensor.matmul(ps2, lhsT=hT[:, j, ms],
                                         rhs=a2w_sb[:, j, :],
                                         start=(j == 0), stop=False)
                    nc.tensor.matmul(ps2, lhsT=ones_row, rhs=a2b_r,
                                     start=False, stop=True)
                    tnh = awork.tile([128, H], F32, tag="aw")
                    nc.scalar.activation(out=tnh, in_=ps2, func=AF.Tanh)
                    nc.vector.tensor_mul(out=sig, in0=sig, in1=tnh)
                    nc.vector.tensor_mul(out=sig, in0=sig, in1=a3w_bc)
                    awc = stat.tile([128, 1], F32, tag="awc")
                    nc.vector.tensor_reduce(out=awc, in_=sig,
                                            axis=mybir.AxisListType.X, op=ALU.add)
                    nc.vector.tensor_add(out=awc, in0=awc, in1=a3b_c)
                    nc.sync.dma_start(out=aw_o.ap()[ms, :], in_=awc)
                    nc.scalar.activation(out=s_cols[:, m:m + 1], in_=awc,
                                         func=AF.Sigmoid)
                    nc.sync.dma_start(out=s_bounce[ms, :], in_=s_cols[:, m:m + 1])

            # ---------------- AllGather s; scalar chain ----------------
            nc.gpsimd.collective_compute(
                "AllGather", ALU.bypass, replica_groups=RG,
                ins=[s_bounce.opt()], outs=[s_full.opt()])

            s_pm = awork.tile([128, N // 128], F32, tag="aw")
            nc.sync.dma_start(
                out=s_pm, in_=s_full.rearrange("(p j) one -> p (j one)", p=128))
            sp_col = stat.tile([128, 1], F32, tag="spc")
            nc.vector.tensor_reduce(out=sp_col, in_=s_pm,
                                    axis=mybir.AxisListType.X, op=ALU.add)
            psS = ps.tile([128, 512], F32, tag="ps")
            nc.tensor.matmul(psS[0:1, 0:1], lhsT=sp_col, rhs=ones_col,
                             start=True, stop=True)
            S_sb = stat.tile([1, 1], F32, tag="Ssb")
            nc.scalar.copy(out=S_sb, in_=psS[0:1, 0:1])
            psSb = ps.tile([128, 512], F32, tag="ps")
            nc.tensor.matmul(psSb[:, 0:1], lhsT=ones_row, rhs=S_sb,
                             start=True, stop=True)
            S_col = stat.tile([128, 1], F32, tag="Scol")
            nc.scalar.copy(out=S_col, in_=psSb[:, 0:1])

            # colsum = s*S - s^2 + 1 ; dinv = rsqrt ; g = s*dinv ; e = dinv^2(1-s^2)
            sS = cols.tile([128, MT], F32)
            nc.vector.tensor_scalar_mul(out=sS, in0=s_cols, scalar1=S_col)
            s2 = cols.tile([128, MT], F32)
            nc.vector.tensor_mul(out=s2, in0=s_cols, in1=s_cols)
            csum = cols.tile([128, MT], F32)
            nc.vector.tensor_sub(out=csum, in0=sS, in1=s2)
            nc.scalar.activation(out=csum, in_=csum, func=AF.Sqrt, bias=1.0)
            dinv = cols.tile([128, MT], F32)
            nc.vector.reciprocal(out=dinv, in_=csum)
            nc.vector.tensor_mul(out=g_cols, in0=s_cols, in1=dinv)
            oms = cols.tile([128, MT], F32)
            nc.scalar.activation(out=oms, in_=s2, func=AF.Copy,
                                 scale=-1.0, bias=1.0)
            d2 = cols.tile([128, MT], F32)
            nc.vector.tensor_mul(out=d2, in0=dinv, in1=dinv)
            nc.vector.tensor_mul(out=e_cols, in0=d2, in1=oms)

            # g in row layout via a dram bounce
            for m in range(MT):
                nc.sync.dma_start(out=g_bounce[m * 128:(m + 1) * 128, :],
                                  in_=g_cols[:, m:m + 1])
            g_row = rows.tile([1, ROWS], F32)
            nc.sync.dma_start(
                out=g_row, in_=g_bounce.rearrange("a b -> b a"))

            with (
                tc.tile_pool(name="sfp", bufs=2) as sfp,
                tc.tile_pool(name="mamp", bufs=3) as mamp,
            ):
                def emit_mam_chunk(c):
                    sfc = sfp.tile([128, MCW], F32, tag="sfc")
                    src = s_full.rearrange("a b -> b a")
                    nc.sync.dma_start(
                        out=sfc,
                        in_=src[:, c * MCW:(c + 1) * MCW].broadcast_to([128, MCW]))
                    for m in range(MT):
                        mc = mamp.tile([128, MCW], F32, tag="mam")
                        if m % 2 == 0:
                            nc.vector.tensor_scalar_mul(out=mc, in0=sfc,
                                                        scalar1=s_cols[:, m:m + 1])
                        else:
                            nc.scalar.activation(out=mc, in_=sfc, func=AF.Copy,
                                                 scale=s_cols[:, m:m + 1])
                        nc.sync.dma_start(
                            out=mam_o.ap()[m * 128:(m + 1) * 128,
                                           c * MCW:(c + 1) * MCW],
                            in_=mc)

                def emit_gcn_layer(li, z_all, zT, w_sb, b_r, lng, lnb, fout,
                                   x_next, xnT, resw_sb=None, resb_r=None):
                    # t = sum_i g_i z_i (own rows), AllReduce across cores
                    pst = ps.tile([128, 512], F32, tag="ps")
                    for m in range(MT):
                        nc.tensor.matmul(pst[0:1, :H], lhsT=g_cols[:, m:m + 1],
                                         rhs=z_all[:, m, :],
                                         start=(m == 0), stop=(m == MT - 1))
                    t_sb = rows.tile([1, H], F32, tag=f"t{li}")
                    nc.scalar.copy(out=t_sb, in_=pst[0:1, :H])
                    nc.sync.dma_start(out=t_bnc[li].opt(), in_=t_sb)
                    nc.gpsimd.collective_compute(
                        "AllReduce", ALU.add, replica_groups=RG,
                        ins=[t_bnc[li].opt()], outs=[t_red[li].opt()])
                    # u = t @ W  ([1, fout])
                    tT = rows.tile([128, KH], F32, tag=f"tT{li}")
                    nc.sync.dma_start(
                        out=tT,
                        in_=t_red[li].rearrange("one (j p) -> p (j one)", p=128))
                    psu = ps.tile([128, 512], F32, tag="ps")
                    for j in range(KH):
                        nc.tensor.matmul(psu[0:1, :fout], lhsT=tT[:, j:j + 1],
                                         rhs=w_sb[:, j, :],
                                         start=(j == 0), stop=(j == KH - 1))
                    u_sb = rows.tile([1, fout], F32, tag=f"u{li}")
                    nc.scalar.copy(out=u_sb, in_=psu[0:1, :fout])

                    for m in range(MT):
                        ms = slice(m * 128, (m + 1) * 128)
                        psV = ps.tile([128, 512], F32, tag="ps")
                        for j in range(KH):
                            nc.tensor.matmul(psV[:, :fout], lhsT=zT[:, j, ms],
                                             rhs=w_sb[:, j, :],
                                             start=(j == 0), stop=(j == KH - 1))
                        psR = ps.tile([128, 512], F32, tag="ps")
                        nc.tensor.matmul(psR[:, :fout], lhsT=g_row[0:1, ms],
                                         rhs=u_sb, start=True, stop=False)
                        nc.tensor.matmul(psR[:, :fout], lhsT=ones_row, rhs=b_r,
                                         start=False, stop=True)
                        if x_next is not None:
                            xn = x_next[:, m, :]
                        else:
                            xw = awork.tile([128, H], F32, tag="aw",
                                            name=f"x3w{m}")
                            xn = xw[:, :fout]
                        nc.vector.tensor_scalar_mul(out=xn, in0=psV[:, :fout],
                                                    scalar1=e_cols[:, m:m + 1])
                        nc.vector.tensor_add(out=xn, in0=xn, in1=psR[:, :fout])
                        nc.scalar.activation(out=xn, in_=xn, func=AF.Relu)
                        _ln_inplace(nc, stat, xn, lng, lnb, eps_col)
                        if resw_sb is None:
                            nc.vector.tensor_add(out=xn, in0=xn,
                                                 in1=z_all[:, m, :])
                            for j in range(KH):
                                pst2 = ps.tile([128, 512], F32, tag="ps")
                                nc.tensor.transpose(
                                    pst2[:, 0:128],
                                    xn[:, j * 128:(j + 1) * 128], ident)
                                nc.scalar.copy(out=xnT[:, j, ms],
                                               in_=pst2[:, 0:128])
                        else:
                            psRes = ps.tile([128, 512], F32, tag="ps")
                            for j in range(KH):
                                nc.tensor.matmul(psRes[:, :OUT], lhsT=zT[:, j, ms],
                                                 rhs=resw_sb[:, j, :],
                                                 start=(j == 0), stop=False)
                            nc.tensor.matmul(psRes[:, :OUT], lhsT=ones_row,
                                             rhs=resb_r, start=False, stop=True)
                            nc.vector.tensor_add(out=xn, in0=xn,
                                                 in1=psRes[:, :OUT])
                            nc.sync.dma_start(out=x3_o.ap()[ms, :], in_=xn)

                x1_all = zg.tile([128, MT, H], F32, tag="zgen")
                x1T = ztp.tile([128, KH, ROWS], F32, tag="zT")
                x2_all = zg.tile([128, MT, H], F32, tag="zgen")
                x2T = ztp.tile([128, KH, ROWS], F32, tag="zT")

                emit_mam_chunk(0)
                emit_gcn_layer(0, h_all, hT, c1w_sb, c1b_r, ln1g_bc, ln1b_bc,
                               H, x1_all, x1T)
                emit_mam_chunk(1)
                emit_gcn_layer(1, x1_all, x1T, c2w_sb, c2b_r, ln2g_bc, ln2b_bc,
                               H, x2_all, x2T)
                emit_mam_chunk(2)
                emit_gcn_layer(2, x2_all, x2T, c3w_sb, c3b_r, ln3g_bc, ln3b_bc,
                               OUT, None, None, resw_sb=resw_sb, resb_r=resb_r)
                emit_mam_chunk(3)

    nc.compile()
    return nc


def kernel(**inputs):
    if "nc" not in _CACHE:
        _CACHE["nc"] = _build()
    nc = _CACHE["nc"]

    f32 = lambda a: np.ascontiguousarray(np.asarray(a), dtype=np.float32)
    x = f32(inputs["x"])
    shared = {}
    for nm in ("p1_w", "p2_w", "a1_w", "a2_w", "c1_w", "c2_w", "c3_w", "res_w"):
        shared[nm] = f32(inputs[nm])
    shared["a3_w"] = f32(inputs["a3_w"]).reshape(1, H)
    for nm, w in (("p1_b", H), ("p2_b", H), ("a1_b", H), ("a2_b", H),
                  ("c1_b", H), ("c2_b", H), ("c3_b", OUT), ("res_b", OUT),
                  ("a3_b", 1),
                  ("ln0_g", H), ("ln0_b", H), ("ln1_g", H), ("ln1_b", H),
                  ("ln2_g", H), ("ln2_b", H), ("ln3_g", OUT), ("ln3_b", OUT)):
        shared[nm] = f32(inputs[nm]).reshape(1, w)

    in_maps = []
    for c in range(NCORES):
        m = dict(shared)
        m["xT"] = np.ascontiguousarray(x[c * ROWS:(c + 1) * ROWS, :].T)
        in_maps.append(m)

    res = run_bass_kernel_spmd(nc, in_maps, core_ids=list(range(NCORES)))

    x3 = np.concatenate([res.results[c]["x3_o"] for c in range(NCORES)], axis=0)
    aw = np.concatenate([res.results[c]["aw_o"] for c in range(NCORES)], axis=0)
    mam = np.concatenate([res.results[c]["mam_o"] for c in range(NCORES)], axis=0)
    return (x3, aw, mam)


# revision 8
# speedup vs baseline: 6405.8060x; 6405.8060x over previous
"""Trainium2 Bass kernel for the GNN message-passing model (nn_NCB_76965813944530).

Math: the reference's N x N "momentum attention matrix" is rank-1
(mam = s s^T, s = sigmoid(aw)), and gcn_norm replaces its diagonal with 1.
So   A = s s^T + diag(1 - s^2),
     colsum_j = s_j (S - s_j) + 1           with S = sum(s)
     dinv = rsqrt(colsum), An = diag(dinv) A diag(dinv)
     An @ M = g (g^T M) + diag(e) M          g = s*dinv, e = dinv^2 (1-s^2)
and  g^T (z @ W) = (g^T z) @ W, so each GCN layer needs only a [1,H]
cross-core AllReduce of t = g^T z instead of an N x N matmul.

Sharding: rows (nodes) split 1024 per core across 8 cores. Weights
replicated. One AllGather of s (for S, and for the mam = s s^T output
rows), then one tiny AllReduce per GCN layer.

The kernel is self-contained: full inputs in, full outputs out.
"""

import os
import sys

for _p in ("/opt/trn_rl_repo", "/root/.axon_site/_ro/trn_rl_repo"):
    if os.path.isdir(_p) and _p not in sys.path:
        sys.path.append(_p)

import numpy as np

import concourse.bass as bass
import concourse.mybir as mybir
import concourse.tile as tile
from concourse import bacc
from concourse.bass_utils import run_bass_kernel_spmd
from concourse.masks import make_identity

N, IN, H, OUT = 8192, 2048, 512, 128
NCORES = 8
ROWS = N // NCORES      # 1024 rows per core
MT = ROWS // 128        # 8 row tiles per core
KIN = IN // 128         # 16 contraction chunks for x @ p1_w
KH = H // 128           # 4 contraction chunks for H-dim matmuls
MAMC = 4                # mam written in 4 column chunks of 2048
MCW = N // MAMC         # 2048
EPS = 1e-5

F32 = mybir.dt.float32
MMDT = mybir.dt.float32  # dtype for DMA-fed matmul operands (float32r = fast)
AF = mybir.ActivationFunctionType
ALU = mybir.AluOpType
RG = [list(range(NCORES))]

_CACHE = {}


def _ln_inplace(nc, stat, t, g_bc, b_bc, eps_col):
    """LayerNorm over the free dim of t ([128, F]), in place, then *g + b."""
    f = t.shape[-1]
    stats = stat.tile([128, 6], F32, tag="st6")
    nc.vector.bn_stats(out=stats, in_=t)
    mv = stat.tile([128, 2], F32, tag="st2")
    nc.vector.bn_aggr(out=mv, in_=stats)
    sd = stat.tile([128, 1], F32, tag="st1")
    nc.scalar.activation(out=sd, in_=mv[:, 1:2], func=AF.Sqrt,
                         bias=eps_col, scale=1.0)
    rs = stat.tile([128, 1], F32, tag="st1b")
    nc.vector.reciprocal(out=rs, in_=sd)
    nc.vector.tensor_scalar(out=t, in0=t, scalar1=mv[:, 0:1], scalar2=rs,
                            op0=ALU.subtract, op1=ALU.mult)
    nc.vector.tensor_mul(out=t, in0=t, in1=g_bc)
    nc.vector.tensor_add(out=t, in0=t, in1=b_bc)


def _build():
    nc = bacc.Bacc("TRN2", num_devices=NCORES, debug=False)

    d_in = {}

    def din(name, shape, dt=MMDT):
        d_in[name] = nc.dram_tensor(name, shape, dt, kind="ExternalInput")
        return d_in[name]

    xT = din("xT", [IN, ROWS])
    p1w = din("p1_w", [IN, H])
    p2w = din("p2_w", [H, H])
    a1w = din("a1_w", [H, H])
    a2w = din("a2_w", [H, H])
    c1w = din("c1_w", [H, H])
    c2w = din("c2_w", [H, H])
    c3w = din("c3_w", [H, OUT])
    resw = din("res_w", [H, OUT])
    a3w = din("a3_w", [1, H], F32)
    p1b = din("p1_b", [1, H])
    p2b = din("p2_b", [1, H])
    a1b = din("a1_b", [1, H])
    a2b = din("a2_b", [1, H])
    c1b = din("c1_b", [1, H])
    c2b = din("c2_b", [1, H])
    c3b = din("c3_b", [1, OUT])
    resb = din("res_b", [1, OUT])
    a3b = din("a3_b", [1, 1], F32)
    ln0g = din("ln0_g", [1, H], F32)
    ln0b = din("ln0_b", [1, H], F32)
    ln1g = din("ln1_g", [1, H], F32)
    ln1b = din("ln1_b", [1, H], F32)
    ln2g = din("ln2_g", [1, H], F32)
    ln2b = din("ln2_b", [1, H], F32)
    ln3g = din("ln3_g", [1, OUT], F32)
    ln3b = din("ln3_b", [1, OUT], F32)

    aw_o = nc.dram_tensor("aw_o", [ROWS, 1], F32, kind="ExternalOutput")
    mam_o = nc.dram_tensor("mam_o", [ROWS, N], F32, kind="ExternalOutput")
    x3_o = nc.dram_tensor("x3_o", [ROWS, OUT], F32, kind="ExternalOutput")

    with tile.TileContext(nc) as tc:
        with (
            tc.tile_pool(name="consts", bufs=1) as consts,
            tc.tile_pool(name="wpool", bufs=1) as wp,
            tc.tile_pool(name="zgen", bufs=2) as zg,
            tc.tile_pool(name="zT", bufs=2) as ztp,
            tc.tile_pool(name="awork", bufs=4) as awork,
            tc.tile_pool(name="stat", bufs=4) as stat,
            tc.tile_pool(name="cols", bufs=1) as cols,
            tc.tile_pool(name="rows", bufs=1) as rows,
            tc.tile_pool(name="ps", bufs=8, space="PSUM") as ps,
            tc.tile_pool(name="dram", bufs=1, space="DRAM") as dram,
        ):
            # ---------------- constants / weights ----------------
            ident = consts.tile([128, 128], F32)
            make_identity(nc, ident)
            ones_row = consts.tile([1, 128], MMDT)
            nc.vector.memset(ones_row, 1.0)
            ones_col = consts.tile([128, 1], F32)
            nc.vector.memset(ones_col, 1.0)
            eps_col = consts.tile([128, 1], F32)
            nc.vector.memset(eps_col, EPS)

            def brow(dt_tensor, f, dt=MMDT):
                t = consts.tile([1, f], dt, tag=f"r_{dt_tensor.name}")
                nc.sync.dma_start(out=t, in_=dt_tensor.ap())
                return t

            def bbc(dt_tensor, f):
                t = consts.tile([128, f], F32, tag=f"b_{dt_tensor.name}")
                nc.sync.dma_start(out=t, in_=dt_tensor.ap().broadcast_to([128, f]))
                return t

            p1b_r = brow(p1b, H)
            p2b_r = brow(p2b, H)
            a1b_r = brow(a1b, H)
            a2b_r = brow(a2b, H)
            c1b_r = brow(c1b, H)
            c2b_r = brow(c2b, H)
            c3b_r = brow(c3b, OUT)
            resb_r = brow(resb, OUT)
            a3w_bc = bbc(a3w, H)
            a3b_c = bbc(a3b, 1)
            ln0g_bc = bbc(ln0g, H)
            ln0b_bc = bbc(ln0b, H)
            ln1g_bc = bbc(ln1g, H)
            ln1b_bc = bbc(ln1b, H)
            ln2g_bc = bbc(ln2g, H)
            ln2b_bc = bbc(ln2b, H)
            ln3g_bc = bbc(ln3g, OUT)
            ln3b_bc = bbc(ln3b, OUT)

            def wload(dt_tensor, fout):
                t = wp.tile([128, KH, fout], MMDT, tag=f"w_{dt_tensor.name}")
                nc.sync.dma_start(
                    out=t, in_=dt_tensor.ap().rearrange("(j p) n -> p j n", p=128))
                return t

            p2w_sb = wload(p2w, H)
            a1w_sb = wload(a1w, H)
            a2w_sb = wload(a2w, H)
            c1w_sb = wload(c1w, H)
            c2w_sb = wload(c2w, H)
            c3w_sb = wload(c3w, OUT)
            resw_sb = wload(resw, OUT)

            # dram bounce buffers for collectives
            s_bounce = dram.tile([ROWS, 1], F32)
            s_full = dram.tile([N, 1], F32)
            g_bounce = dram.tile([ROWS, 1], F32)
            t_bnc = [dram.tile([1, H], F32, tag=f"tb{i}", name=f"t_bnc{i}")
                     for i in range(3)]
            t_red = [dram.tile([1, H], F32, tag=f"tr{i}", name=f"t_red{i}")
                     for i in range(3)]

            # per-row column vectors, one column per m-tile
            s_cols = cols.tile([128, MT], F32)
            g_cols = cols.tile([128, MT], F32)
            e_cols = cols.tile([128, MT], F32)

            h_all = zg.tile([128, MT, H], F32, tag="zgen")

            # ---------------- stage A: x @ p1_w (+b, relu, LN0) ----------------
            psA = [ps.tile([128, 512], F32, tag="ps", name=f"psA{m}")
                   for m in range(MT)]
            with (
                tc.tile_pool(name="xa", bufs=4) as xa,
                tc.tile_pool(name="p1p", bufs=4) as p1p,
            ):
                for k in range(KIN):
                    xt = xa.tile([128, ROWS], MMDT, tag="xt")
                    nc.sync.dma_start(out=xt, in_=xT.ap()[k * 128:(k + 1) * 128, :])
                    wt = p1p.tile([128, H], MMDT, tag="p1w")
                    nc.sync.dma_start(out=wt, in_=p1w.ap()[k * 128:(k + 1) * 128, :])
                    for m in range(MT):
                        nc.tensor.matmul(psA[m], lhsT=xt[:, m * 128:(m + 1) * 128],
                                         rhs=wt, start=(k == 0), stop=False)
                for m in range(MT):
                    nc.tensor.matmul(psA[m], lhsT=ones_row, rhs=p1b_r,
                                     start=False, stop=True)

                lnh0T = ztp.tile([128, KH, ROWS], F32, tag="zT")
                hT = ztp.tile([128, KH, ROWS], F32, tag="zT")

                for m in range(MT):
                    ms = slice(m * 128, (m + 1) * 128)
                    # bias already in psum; relu copies psum -> sbuf
                    lnh0 = awork.tile([128, H], F32, tag="aw")
                    nc.scalar.activation(out=lnh0, in_=psA[m], func=AF.Relu)
                    _ln_inplace(nc, stat, lnh0, ln0g_bc, ln0b_bc, eps_col)
                    for j in range(KH):
                        pst = ps.tile([128, 512], F32, tag="ps")
                        nc.tensor.transpose(pst[:, 0:128],
                                            lnh0[:, j * 128:(j + 1) * 128], ident)
                        nc.scalar.copy(out=lnh0T[:, j, ms], in_=pst[:, 0:128])

                    # ---------------- stage B: h = LN0 @ p2_w + b ----------------
                    psB = ps.tile([128, 512], F32, tag="ps")
                    for j in range(KH):
                        nc.tensor.matmul(psB, lhsT=lnh0T[:, j, ms],
                                         rhs=p2w_sb[:, j, :],
                                         start=(j == 0), stop=False)
                    nc.tensor.matmul(psB, lhsT=ones_row, rhs=p2b_r,
                                     start=False, stop=True)
                    hcol = h_all[:, m, :]
                    nc.scalar.copy(out=hcol, in_=psB)
                    for j in range(KH):
                        pst = ps.tile([128, 512], F32, tag="ps")
                        nc.tensor.transpose(pst[:, 0:128],
                                            hcol[:, j * 128:(j + 1) * 128], ident)
                        nc.scalar.copy(out=hT[:, j, ms], in_=pst[:, 0:128])

                    # ---------------- attention gate -> aw, s ----------------
                    ps1 = ps.tile([128, 512], F32, tag="ps")
                    for j in range(KH):
                        nc.tensor.matmul(ps1, lhsT=hT[:, j, ms],
                                         rhs=a1w_sb[:, j, :],
                                         start=(j == 0), stop=False)
                    nc.tensor.matmul(ps1, lhsT=ones_row, rhs=a1b_r,
                                     start=False, stop=True)
                    sig = awork.tile([128, H], F32, tag="aw")
                    nc.scalar.activation(out=sig, in_=ps1, func=AF.Sigmoid)
                    ps2 = ps.tile([128, 512], F32, tag="ps")
                    for j in range(KH):
                        nc.tensor.matmul(ps2, lhsT=hT[:, j, ms],
                                         rhs=a2w_sb[:, j, :],
                                         start=(j == 0), stop=False)
                    nc.tensor.matmul(ps2, lhsT=ones_row, rhs=a2b_r,
                                     start=False, stop=True)
                    tnh = awork.tile([128, H], F32, tag="aw")
                    nc.scalar.activation(out=tnh, in_=ps2, func=AF.Tanh)
                    nc.vector.tensor_mul(out=sig, in0=sig, in1=tnh)
                    nc.vector.tensor_mul(out=sig, in0=sig, in1=a3w_bc)
                    awc = stat.tile([128, 1], F32, tag="awc")
                    nc.vector.tensor_reduce(out=awc, in_=sig,
                                            axis=mybir.AxisListType.X, op=ALU.add)
                    nc.vector.tensor_add(out=awc, in0=awc, in1=a3b_c)
                    nc.sync.dma_start(out=aw_o.ap()[ms, :], in_=awc)
                    nc.scalar.activation(out=s_cols[:, m:m + 1], in_=awc,
                                         func=AF.Sigmoid)
                    nc.sync.dma_start(out=s_bounce[ms, :], in_=s_cols[:, m:m + 1])

            # ---------------- AllGather s; scalar chain ----------------
            nc.gpsimd.collective_compute(
                "AllGather", ALU.bypass, replica_groups=RG,
                ins=[s_bounce.opt()], outs=[s_full.opt()])

            s_pm = awork.tile([128, N // 128], F32, tag="aw")
            nc.sync.dma_start(
                out=s_pm, in_=s_full.rearrange("(p j) one -> p (j one)", p=128))
            sp_col = stat.tile([128, 1], F32, tag="spc")
            nc.vector.tensor_reduce(out=sp_col, in_=s_pm,
                                    axis=mybir.AxisListType.X, op=ALU.add)
            psS = ps.tile([128, 512], F32, tag="ps")
            nc.tensor.matmul(psS[0:1, 0:1], lhsT=sp_col, rhs=ones_col,
                             start=True, stop=True)
            S_sb = stat.tile([1, 1], F32, tag="Ssb")
            nc.scalar.copy(out=S_sb, in_=psS[0:1, 0:1])
            psSb = ps.tile([128, 512], F32, tag="ps")
            nc.tensor.matmul(psSb[:, 0:1], lhsT=ones_row, rhs=S_sb,
                             start=True, stop=True)
            S_col = stat.tile([128, 1], F32, tag="Scol")
            nc.scalar.copy(out=S_col, in_=psSb[:, 0:1])

            # colsum = s*S - s^2 + 1 ; dinv = rsqrt ; g = s*dinv ; e = dinv^2(1-s^2)
            sS = cols.tile([128, MT], F32)
            nc.vector.tensor_scalar_mul(out=sS, in0=s_cols, scalar1=S_col)
            s2 = cols.tile([128, MT], F32)
            nc.vector.tensor_mul(out=s2, in0=s_cols, in1=s_cols)
            csum = cols.tile([128, MT], F32)
            nc.vector.tensor_sub(out=csum, in0=sS, in1=s2)
            nc.scalar.activation(out=csum, in_=csum, func=AF.Sqrt, bias=1.0)
            dinv = cols.tile([128, MT], F32)
            nc.vector.reciprocal(out=dinv, in_=csum)
            nc.vector.tensor_mul(out=g_cols, in0=s_cols, in1=dinv)
            oms = cols.tile([128, MT], F32)
            nc.scalar.activation(out=oms, in_=s2, func=AF.Copy,
                                 scale=-1.0, bias=1.0)
            d2 = cols.tile([128, MT], F32)
            nc.vector.tensor_mul(out=d2, in0=dinv, in1=dinv)
            nc.vector.tensor_mul(out=e_cols, in0=d2, in1=oms)

            # g in row layout via a dram bounce
            for m in range(MT):
                nc.sync.dma_start(out=g_bounce[m * 128:(m + 1) * 128, :],
                                  in_=g_cols[:, m:m + 1])
            g_row = rows.tile([1, ROWS], F32)
            nc.sync.dma_start(
                out=g_row, in_=g_bounce.rearrange("a b -> b a"))

            with (
                tc.tile_pool(name="sfp", bufs=2) as sfp,
                tc.tile_pool(name="mamp", bufs=3) as mamp,
            ):
                def emit_mam_chunk(c):
                    sfc = sfp.tile([128, MCW], F32, tag="sfc")
                    src = s_full.rearrange("a b -> b a")
                    nc.sync.dma_start(
                        out=sfc,
                        in_=src[:, c * MCW:(c + 1) * MCW].broadcast_to([128, MCW]))
                    for m in range(MT):
                        mc = mamp.tile([128, MCW], F32, tag="mam")
                        if m % 2 == 0:
                            nc.vector.tensor_scalar_mul(out=mc, in0=sfc,
                                                        scalar1=s_cols[:, m:m + 1])
                        else:
                            nc.scalar.activation(out=mc, in_=sfc, func=AF.Copy,
                                                 scale=s_cols[:, m:m + 1])
                        nc.sync.dma_start(
                            out=mam_o.ap()[m * 128:(m + 1) * 128,
                                           c * MCW:(c + 1) * MCW],
                            in_=mc)

                def emit_gcn_layer(li, z_all, zT, w_sb, b_r, lng, lnb, fout,
                                   x_next, xnT, resw_sb=None, resb_r=None):
                    # t = sum_i g_i z_i (own rows), AllReduce across cores
                    pst = ps.tile([128, 512], F32, tag="ps")
                    for m in range(MT):
                        nc.tensor.matmul(pst[0:1, :H], lhsT=g_cols[:, m:m + 1],
                                         rhs=z_all[:, m, :],
                                         start=(m == 0), stop=(m == MT - 1))
                    t_sb = rows.tile([1, H], F32, tag=f"t{li}")
                    nc.scalar.copy(out=t_sb, in_=pst[0:1, :H])
                    nc.sync.dma_start(out=t_bnc[li].opt(), in_=t_sb)
                    nc.gpsimd.collective_compute(
                        "AllReduce", ALU.add, replica_groups=RG,
                        ins=[t_bnc[li].opt()], outs=[t_red[li].opt()])
                    # u = t @ W  ([1, fout])
                    tT = rows.tile([128, KH], F32, tag=f"tT{li}")
                    nc.sync.dma_start(
                        out=tT,
                        in_=t_red[li].rearrange("one (j p) -> p (j one)", p=128))
                    psu = ps.tile([128, 512], F32, tag="ps")
                    for j in range(KH):
                        nc.tensor.matmul(psu[0:1, :fout], lhsT=tT[:, j:j + 1],
                                         rhs=w_sb[:, j, :],
                                         start=(j == 0), stop=(j == KH - 1))
                    u_sb = rows.tile([1, fout], F32, tag=f"u{li}")
                    nc.scalar.copy(out=u_sb, in_=psu[0:1, :fout])

                    for m in range(MT):
                        ms = slice(m * 128, (m + 1) * 128)
                        psV = ps.tile([128, 512], F32, tag="ps")
                        for j in range(KH):
                            nc.tensor.matmul(psV[:, :fout], lhsT=zT[:, j, ms],
                                             rhs=w_sb[:, j, :],
                                             start=(j == 0), stop=(j == KH - 1))
                        psR = ps.tile([128, 512], F32, tag="ps")
                        nc.tensor.matmul(psR[:, :fout], lhsT=g_row[0:1, ms],
                                         rhs=u_sb, start=True, stop=False)
                        nc.tensor.matmul(psR[:, :fout], lhsT=ones_row, rhs=b_r,
                                         start=False, stop=True)
                        if x_next is not None:
                            xn = x_next[:, m, :]
                        else:
                            xw = awork.tile([128, H], F32, tag="aw",
                                            name=f"x3w{m}")
                            xn = xw[:, :fout]
                        nc.vector.tensor_scalar_mul(out=xn, in0=psV[:, :fout],
                                                    scalar1=e_cols[:, m:m + 1])
                        nc.vector.tensor_add(out=xn, in0=xn, in1=psR[:, :fout])
                        nc.scalar.activation(out=xn, in_=xn, func=AF.Relu)
                        _ln_inplace(nc, stat, xn, lng, lnb, eps_col)
                        if resw_sb is None:
                            nc.vector.tensor_add(out=xn, in0=xn,
                                                 in1=z_all[:, m, :])
                            for j in range(KH):
                                pst2 = ps.tile([128, 512], F32, tag="ps")
                                nc.tensor.transpose(
                                    pst2[:, 0:128],
                                    xn[:, j * 128:(j + 1) * 128], ident)
                                nc.scalar.copy(out=xnT[:, j, ms],
                                               in_=pst2[:, 0:128])
                        else:
                            psRes = ps.tile([128, 512], F32, tag="ps")
                            for j in range(KH):
                                nc.tensor.matmul(psRes[:, :OUT], lhsT=zT[:, j, ms],
                                                 rhs=resw_sb[:, j, :],
                                                 start=(j == 0), stop=False)
                            nc.tensor.matmul(psRes[:, :OUT], lhsT=ones_row,
                                             rhs=resb_r, start=False, stop=True)
                            nc.vector.tensor_add(out=xn, in0=xn,
                                                 in1=psRes[:, :OUT])
                            nc.sync.dma_start(out=x3_o.ap()[ms, :], in_=xn)

                x1_all = zg.tile([128, MT, H], F32, tag="zgen")
                x1T = ztp.tile([128, KH, ROWS], F32, tag="zT")
                x2_all = zg.tile([128, MT, H], F32, tag="zgen")
                x2T = ztp.tile([128, KH, ROWS], F32, tag="zT")

                emit_mam_chunk(0)
                emit_gcn_layer(0, h_all, hT, c1w_sb, c1b_r, ln1g_bc, ln1b_bc,
                               H, x1_all, x1T)
                emit_mam_chunk(1)
                emit_gcn_layer(1, x1_all, x1T, c2w_sb, c2b_r, ln2g_bc, ln2b_bc,
                               H, x2_all, x2T)
                emit_mam_chunk(2)
                emit_gcn_layer(2, x2_all, x2T, c3w_sb, c3b_r, ln3g_bc, ln3b_bc,
                               OUT, None, None, resw_sb=resw_sb, resb_r=resb_r)
                emit_mam_chunk(3)

    nc.compile()
    return nc


def _get_runner():
    """Build (once) a cached jitted SPMD runner for the compiled bass module.

    Mirrors bass2jax.run_bass_via_pjrt's multi-core branch, but keeps the
    jitted callable so repeated kernel() calls skip re-tracing/lowering.
    """
    if "runner" in _CACHE:
        return _CACHE["runner"]

    import jax
    from jax.sharding import Mesh, PartitionSpec
    from jax.experimental.shard_map import shard_map
    from concourse import bass2jax, mybir as _mybir

    nc = _build()
    bass2jax.install_neuronx_cc_hook()

    partition_name = (nc.partition_id_tensor.name
                      if nc.partition_id_tensor else None)
    in_names, out_names, out_avals, zero_shapes = [], [], [], []
    for alloc in nc.m.functions[0].allocations:
        if not isinstance(alloc, _mybir.MemoryLocationSet):
            continue
        name = alloc.memorylocations[0].name
        if alloc.kind == "ExternalInput":
            if name != partition_name:
                in_names.append(name)
        elif alloc.kind == "ExternalOutput":
            shape = tuple(alloc.tensor_shape)
            dtype = _mybir.dt.np(alloc.dtype)
            out_names.append(name)
            out_avals.append(jax.core.ShapedArray(shape, dtype))
            zero_shapes.append((shape, dtype))
    n_params = len(in_names)
    n_outs = len(out_avals)
    all_in_names = list(in_names) + list(out_names)
    if partition_name is not None:
        all_in_names.append(partition_name)
    donate = tuple(range(n_params, n_params + n_outs))

    def _body(*args):
        operands = list(args)
        if partition_name is not None:
            operands.append(bass2jax.partition_id_tensor())
        outs = bass2jax._bass_exec_p.bind(
            *operands,
            out_avals=tuple(out_avals),
            in_names=tuple(all_in_names),
            out_names=tuple(out_names),
            lowering_input_output_aliases=(),
            sim_require_finite=True,
            sim_require_nnan=True,
            nc=nc,
        )
        return tuple(outs)

    devices = jax.devices()[:NCORES]
    assert len(devices) == NCORES
    mesh = Mesh(np.asarray(devices), ("core",))
    in_specs = (PartitionSpec("core"),) * (n_params + n_outs)
    out_specs = (PartitionSpec("core"),) * n_outs
    fn = jax.jit(
        shard_map(_body, mesh=mesh, in_specs=in_specs, out_specs=out_specs,
                  check_rep=False),
        donate_argnums=donate, keep_unused=True)

    runner = {
        "fn": fn, "in_names": in_names, "out_names": out_names,
        "zero_shapes": zero_shapes, "mesh": mesh,
    }
    _CACHE["runner"] = runner
    return runner


def _marshal(inputs):
    """FULL inputs -> concatenated per-core global arrays keyed by in_name."""
    f32 = lambda a: np.ascontiguousarray(np.asarray(a), dtype=np.float32)
    x = f32(inputs["x"])
    shared = {}
    for nm in ("p1_w", "p2_w", "a1_w", "a2_w", "c1_w", "c2_w", "c3_w", "res_w"):
        shared[nm] = f32(inputs[nm])
    shared["a3_w"] = f32(inputs["a3_w"]).reshape(1, H)
    for nm, w in (("p1_b", H), ("p2_b", H), ("a1_b", H), ("a2_b", H),
                  ("c1_b", H), ("c2_b", H), ("c3_b", OUT), ("res_b", OUT),
                  ("a3_b", 1),
                  ("ln0_g", H), ("ln0_b", H), ("ln1_g", H), ("ln1_b", H),
                  ("ln2_g", H), ("ln2_b", H), ("ln3_g", OUT), ("ln3_b", OUT)):
        shared[nm] = f32(inputs[nm]).reshape(1, w)

    concat = {}
    # xT differs per core; everything else is replicated 8x along axis 0
    xt_full = np.empty((NCORES * IN, ROWS), np.float32)
    for c in range(NCORES):
        np.copyto(xt_full[c * IN:(c + 1) * IN], x[c * ROWS:(c + 1) * ROWS, :].T)
    concat["xT"] = xt_full
    for nm, arr in shared.items():
        concat[nm] = np.ascontiguousarray(
            np.broadcast_to(arr[None], (NCORES,) + arr.shape)
            .reshape((NCORES * arr.shape[0],) + arr.shape[1:]))
    return concat


def _invoke(concat):
    r = _get_runner()
    args = [concat[nm] for nm in r["in_names"]]
    zeros = [np.zeros((NCORES * s[0],) + tuple(s[1:]), dt)
             for s, dt in r["zero_shapes"]]
    outs = r["fn"](*args, *zeros)
    return dict(zip(r["out_names"], outs))


def kernel(**inputs):
    concat = _marshal(inputs)
    outs = _invoke(concat)
    x3 = np.asarray(outs["x3_o"]).reshape(N, OUT)
    aw = np.asarray(outs["aw_o"]).reshape(N, 1)
    mam = np.asarray(outs["mam_o"]).reshape(N, N)
    return (x3, aw, mam)
